# revision 1
# baseline (speedup 1.0000x reference)
"""Causal self-attention (RoPE) Trainium2 kernel.

Problem: B=4, T=2048, D=1024, H=16 heads (hd=64), fp32.
  q,k,v = x@W{q,k,v}.T + b;  rope(q), rope(k);  causal softmax attention;
  y = att_out @ Wo.T + bo.

Sharding (8 cores): data parallel over batch (4), tensor parallel over
heads (2 groups of 8 heads). Core c handles batch c//2, head-group c%2.
Each core computes its 8 heads end-to-end plus the partial out-projection;
the host sums the two head-group partials per batch and adds bo.

On-device layout is transposed ([dim, time]) so that attention matmuls get
the contraction dim (head dim / keys) on partitions:
  - x.T built via PE transposes
  - Q.T/K.T = W @ x.T directly; RoPE applied elementwise, with the
    rotate-half realized as a single +/-1 permutation-matrix matmul on PE
  - V projected per x-strip, interleaved with the transposes (PE filler
    while the next strip's DMA is in flight)
  - S.T = K_h @ Q_h.T per head-PAIR (even head at partitions 0-63, odd at
    64-127 — the two K=64 matmuls run concurrently in the PE array); one
    exp per pair over [128, 1024], diagonal tiles sliced to the valid
    region with a static [128,128] 0/-1e30 triangle added to the S-PSUM
    before exp (keeps at-tiles pure ACT->PE, no event-semaphore storms)
  - O.T = [V_h | 1].T @ A.T accumulated over key tiles; the ones column
    yields the softmax denominator as psum row 64; AV matmuls trail the
    S matmuls by LAG key-tile pairs, and each pair's trailing AVs,
    reciprocals and normalization tails are deferred into the next pair's
    instruction stream so the PE FIFO never drains at pair boundaries
  - normalization via reciprocal + K=1 broadcast matmul
  - out.T = Wo_c.T @ Y.T, emitted per finished query chunk (interleaved
    into the next chunk's attention), written transposed; host transposes

All big matmuls run in float32r (full-rate fp32, ~1e-4 relative rounding).
"""

import sys

sys.path.insert(0, "/opt/trn_rl_repo")

import numpy as np

B, T, D, H = 4, 2048, 1024, 16
HD = 64
ROPE_BASE = 10000.0
N_CORES = 8
HPC = 8  # heads per core
LAG = 5  # AV matmul lag behind S matmul (key-tile pairs)

_cache = {}


def _build_bass():
    import concourse.mybir as mybir
    import concourse.tile as tile
    from concourse import bacc

    f32 = mybir.dt.float32
    f32r = mybir.dt.float32r
    Alu = mybir.AluOpType
    Act = mybir.ActivationFunctionType

    nc = bacc.Bacc()

    # ---- DRAM I/O (per-core shards; same NEFF on all 8 cores) ----
    x_d = nc.dram_tensor("x", [T, D], f32r, kind="ExternalInput")
    wq_d = nc.dram_tensor("wq", [D, 512], f32r, kind="ExternalInput")
    wk_d = nc.dram_tensor("wk", [D, 512], f32r, kind="ExternalInput")
    wv_d = nc.dram_tensor("wv", [D, 512], f32r, kind="ExternalInput")
    wo_d = nc.dram_tensor("wo", [512, D], f32r, kind="ExternalInput")
    bq_d = nc.dram_tensor("bq", [128, 4], f32, kind="ExternalInput")
    bk_d = nc.dram_tensor("bk", [128, 4], f32, kind="ExternalInput")
    bv_d = nc.dram_tensor("bv_bc", [128, 512], f32, kind="ExternalInput")
    cos_d = nc.dram_tensor("cosT", [128, T], f32r, kind="ExternalInput")
    sin_d = nc.dram_tensor("sinT", [128, T], f32r, kind="ExternalInput")
    perm_d = nc.dram_tensor("permT", [128, 128], f32r, kind="ExternalInput")
    tri_d = nc.dram_tensor("triadd", [128, 128], f32, kind="ExternalInput")
    ident_d = nc.dram_tensor("ident", [128, 128], f32r, kind="ExternalInput")
    vones_d = nc.dram_tensor("vones", [128, 16, 8], f32r, kind="ExternalInput")
    yt_d = nc.dram_tensor("yT", [D, T], f32, kind="ExternalOutput")

    wq_v = wq_d[:, :].rearrange("(ko p) m -> p ko m", p=128)
    wk_v = wk_d[:, :].rearrange("(ko p) m -> p ko m", p=128)
    wv_v = wv_d[:, :].rearrange("(ko p) m -> p ko m", p=128)
    wo_v = wo_d[:, :].rearrange("(ko p) n -> p ko n", p=128)

    with tile.TileContext(nc) as tc:
        with (
            tc.tile_pool(name="singles", bufs=1) as singles,
            tc.tile_pool(name="big", bufs=1) as big,
        ):
            ident = singles.tile([128, 128], f32r)
            nc.sync.dma_start(ident, ident_d[:, :])
            ones_hi = singles.tile([65, 64], f32)
            nc.vector.memset(ones_hi, 1.0)
            bq_sb = singles.tile([128, 4], f32, tag="bq")
            nc.sync.dma_start(bq_sb, bq_d[:, :])
            bk_sb = singles.tile([128, 4], f32, tag="bk")
            nc.sync.dma_start(bk_sb, bk_d[:, :])
            perm_sb = singles.tile([128, 128], f32r, tag="perm")
            nc.sync.dma_start(perm_sb, perm_d[:, :])
            tri_sb = singles.tile([128, 128], f32, tag="tri")
            nc.sync.dma_start(tri_sb, tri_d[:, :])

            # persistent activations (f32r so matmuls can consume them)
            qt = big.tile([128, 4, T], f32r, tag="qt")
            kt = big.tile([128, 4, T], f32r, tag="kt")
            v_sb = big.tile([128, 16, HPC, 65], f32r, tag="v")
            nc.sync.dma_start(
                v_sb[:, :, :, 64:65], vones_d[:, :, :][:, :, :, None])

            # ================= Phase A: x.T, Q.T/K.T (roped), V =============
            with (
                tc.tile_pool(name="pa_sb", bufs=1) as pa,
                tc.tile_pool(name="xrow_p", bufs=4) as xrow_p,
                tc.tile_pool(name="wcol_p", bufs=2) as wcol_p,
                tc.tile_pool(name="qa_p", bufs=4) as qa_p,
                tc.tile_pool(name="tmp_p", bufs=2) as tmp_p,
                tc.tile_pool(name="tps", bufs=2, space="PSUM") as tps,
                tc.tile_pool(name="qkps", bufs=3, space="PSUM") as qkps,
                tc.tile_pool(name="auxps", bufs=3, space="PSUM") as auxps,
            ):
                # wv first: the V matmuls of strip 0 sit early in the PE
                # FIFO and must not wait behind the cos/sin table DMAs
                wv_sb = pa.tile([128, 8, 512], f32r, tag="wv")
                nc.scalar.dma_start(wv_sb, wv_v)
                bv_sb = pa.tile([128, 512], f32, tag="bv")
                nc.scalar.dma_start(bv_sb, bv_d[:, :])
                cos_sb = pa.tile([128, T], f32r, tag="cos")
                nc.scalar.dma_start(cos_sb, cos_d[:, :])
                sin_sb = pa.tile([128, T], f32r, tag="sin")
                nc.scalar.dma_start(sin_sb, sin_d[:, :])

                for th in range(2):
                    t0 = th * 1024
                    xt = pa.tile([128, 8, 1024], f32r, tag="xt")
                    # --- A1: transpose x strips into xt; V projection of
                    # each strip follows immediately (PE filler while the
                    # next strip's DMA is in flight) ---
                    for tt in range(8):
                        r0 = t0 + tt * 128
                        xrow = xrow_p.tile([128, D], f32r, tag="xrow")
                        nc.sync.dma_start(xrow, x_d[r0:r0 + 128, :])
                        for ko in range(8):
                            tp = tps.tile([128, 128], f32r, tag="tps")
                            nc.tensor.transpose(
                                tp, xrow[:, ko * 128:(ko + 1) * 128], ident)
                            nc.scalar.copy(
                                xt[:, ko, tt * 128:(tt + 1) * 128], tp)
                        gt = th * 8 + tt
                        psV = auxps.tile([128, 512], f32, tag="aux",
                                         name=f"psV_{gt}")
                        for ko in range(8):
                            nc.tensor.matmul(
                                psV, lhsT=xt[:, ko, tt * 128:(tt + 1) * 128],
                                rhs=wv_sb[:, ko, :],
                                start=(ko == 0), stop=(ko == 7))
                        nc.vector.tensor_tensor(
                            v_sb[:, gt, :, 0:64],
                            psV.rearrange("p (h d) -> p h d", h=HPC),
                            bv_sb.rearrange("p (h d) -> p h d", h=HPC),
                            Alu.add)

                    # --- A2: Q.T and K.T with fused RoPE ---
                    for wview, bcol, dest in (
                        (wq_v, bq_sb, qt),
                        (wk_v, bk_sb, kt),
                    ):
                        for qo in range(4):
                            wcol = wcol_p.tile([128, 8, 128], f32r, tag="wcol")
                            nc.sync.dma_start(
                                wcol, wview[:, :, qo * 128:(qo + 1) * 128])
                            # two 512-chunks in flight so the rot matmul's
                            # DVE dependency hides under the other chunk's
                            # accumulation matmuls
                            ps_l, qa_l = [], []
                            for tcc in range(2):
                                c0 = tcc * 512
                                psA = qkps.tile([128, 512], f32, tag="psA")
                                for ko in range(8):
                                    nc.tensor.matmul(
                                        psA, lhsT=wcol[:, ko, :],
                                        rhs=xt[:, ko, c0:c0 + 512],
                                        start=(ko == 0), stop=(ko == 7))
                                qa = qa_p.tile([128, 512], f32r, tag="qa")
                                nc.vector.tensor_scalar_add(
                                    qa, psA, bcol[:, qo:qo + 1])
                                ps_l.append(psA)
                                qa_l.append(qa)
                            rot_l = []
                            for tcc in range(2):
                                rps = auxps.tile([128, 512], f32, tag="aux")
                                nc.tensor.matmul(
                                    rps, lhsT=perm_sb, rhs=qa_l[tcc],
                                    start=True, stop=True)
                                rot_l.append(rps)
                            for tcc in range(2):
                                ta = t0 + tcc * 512
                                tmp1 = tmp_p.tile([128, 512], f32, tag="tmp1")
                                nc.vector.tensor_tensor(
                                    tmp1, qa_l[tcc], cos_sb[:, ta:ta + 512],
                                    Alu.mult)
                                tmp2 = tmp_p.tile([128, 512], f32, tag="tmp2")
                                nc.vector.tensor_tensor(
                                    tmp2, rot_l[tcc], sin_sb[:, ta:ta + 512],
                                    Alu.mult)
                                nc.vector.tensor_tensor(
                                    dest[:, qo, ta:ta + 512], tmp1, tmp2,
                                    Alu.add)


            # ================= Phase B: attention ==========================
            with tc.tile_pool(name="pb_keep", bufs=1) as pb_keep:
                yt = pb_keep.tile([128, 4, T], f32r, tag="yt")
                wo_sb = pb_keep.tile([128, 4, D], f32r, tag="wo")
                nc.scalar.dma_start(wo_sb, wo_v)
                with (
                    tc.tile_pool(name="at_p", bufs=7) as at_p,
                    tc.tile_pool(name="rec_p", bufs=4) as rec_p,
                    tc.tile_pool(name="ytmp_p", bufs=2) as ytmp_p,
                    tc.tile_pool(name="orow_p", bufs=4) as orow_p,
                    tc.tile_pool(name="sps", bufs=2, space="PSUM") as sps,
                    tc.tile_pool(name="ops", bufs=4, space="PSUM") as ops,
                ):
                    def emit_outproj(cj, dos=range(8)):
                        p0 = cj * 512
                        for do in dos:
                            ps2 = sps.tile([128, 1024], f32, tag="sps",
                                           name=f"op_{cj}_{do}")
                            ps = ps2[:, 0:512]
                            for ko in range(4):
                                nc.tensor.matmul(
                                    ps, lhsT=wo_sb[:, ko,
                                                   do * 128:(do + 1) * 128],
                                    rhs=yt[:, ko, p0:p0 + 512],
                                    start=(ko == 0), stop=(ko == 3))
                            orow = orow_p.tile([128, 512], f32, tag="orow")
                            nc.vector.tensor_copy(orow, ps)
                            nc.sync.dma_start(
                                yt_d[do * 128:(do + 1) * 128, p0:p0 + 512],
                                orow)

                    pending = []
                    pending_avs = []
                    pending_recips = []

                    def flush_avs():
                        for fn in pending_avs:
                            fn()
                        pending_avs.clear()
                        for fn in pending_recips:
                            fn()
                        pending_recips.clear()

                    def flush_tails():
                        flush_avs()
                        for fn in pending:
                            fn()
                        pending.clear()

                    for ci in range(4):
                        q0 = ci * 512
                        nkt = 4 * ci + 4
                        for ho in range(4):
                            if ho == 1 and ci > 0:
                                flush_tails()
                                emit_outproj(ci - 1, range(0, 4))
                            elif ho == 2 and ci > 0:
                                emit_outproj(ci - 1, range(4, 8))
                            o_pair = [
                                ops.tile([65, 512], f32, tag="ops",
                                         name=f"ops_{ci}_{ho}_{g_}")
                                for g_ in range(2)]
                            at2s = {}

                            def do_av(k_i, o_pair=o_pair, at2s=at2s, nkt=nkt,
                                      ho=ho, ci=ci):
                                at2 = at2s.pop(k_i)
                                sdx = k_i - 4 * ci
                                f0 = max(0, sdx) * 128
                                for g in range(2):
                                    nc.tensor.matmul(
                                        o_pair[g][:, f0:512],
                                        lhsT=v_sb[:, k_i, 2 * ho + g, :],
                                        rhs=at2[:, g * 512 + f0:
                                                (g + 1) * 512],
                                        start=(k_i == 0),
                                        stop=(k_i == nkt - 1))

                            for kt_i in range(nkt):
                                sdx = kt_i - 4 * ci
                                f0 = max(0, sdx) * 128
                                s_ps2 = sps.tile([128, 1024], f32, tag="sps")
                                for g in range(2):
                                    hp = g * 64
                                    nc.tensor.matmul(
                                        s_ps2[:, g * 512 + f0:(g + 1) * 512],
                                        lhsT=kt[hp:hp + 64, ho,
                                                kt_i * 128:(kt_i + 1) * 128],
                                        rhs=qt[hp:hp + 64, ho,
                                               q0 + f0:q0 + 512],
                                        start=True, stop=True)
                                at2 = at_p.tile([128, 1024], f32r, tag="at")
                                av = at2.rearrange("p (g q) -> p g q", g=2)
                                sv = s_ps2.rearrange("p (g q) -> p g q", g=2)
                                if sdx >= 0:
                                    nc.vector.tensor_tensor(
                                        sv[:, :, f0:f0 + 128],
                                        sv[:, :, f0:f0 + 128],
                                        tri_sb[:, None, :].to_broadcast(
                                            (128, 2, 128)),
                                        Alu.add)
                                nc.scalar.activation(
                                    av[:, :, f0:512], sv[:, :, f0:512],
                                    Act.Exp, scale=0.125)
                                at2s[kt_i] = at2
                                if kt_i == 0:
                                    flush_avs()
                                elif kt_i == 1:
                                    flush_tails()
                                if kt_i >= LAG:
                                    do_av(kt_i - LAG)
                            for k_i in range(max(0, nkt - LAG), nkt):
                                pending_avs.append(
                                    lambda k_i=k_i, do_av=do_av: do_av(k_i))

                            recs = [rec_p.tile([65, 512], f32, tag="rec",
                                               name=f"rec_{ci}_{ho}_{g_}")
                                    for g_ in range(2)]
                            for g in range(2):
                                o_ps = o_pair[g]
                                rec = recs[g]
                                pending_recips.append(
                                    lambda o_ps=o_ps, rec=rec:
                                    nc.vector.reciprocal(
                                        rec[64:65, :], o_ps[64:65, :]))

                                def rest(g=g, o_ps=o_ps, rec=rec, ho=ho,
                                         q0=q0, ci=ci):
                                    b_ps = ops.tile(
                                        [64, 512], f32, tag="ops",
                                        name=f"bps_{ci}_{ho}_{g}")
                                    nc.tensor.matmul(
                                        b_ps, lhsT=ones_hi[64:65, :],
                                        rhs=rec[64:65, :],
                                        start=True, stop=True)
                                    b_sb2 = rec_p.tile(
                                        [64, 512], f32, tag="bsb")
                                    nc.vector.tensor_copy(b_sb2, b_ps)
                                    if g == 0:
                                        nc.vector.tensor_tensor(
                                            yt[0:64, ho, q0:q0 + 512],
                                            o_ps[0:64, :], b_sb2, Alu.mult)
                                    else:
                                        ytmp = ytmp_p.tile(
                                            [64, 512], f32r, tag="ytmp")
                                        nc.vector.tensor_tensor(
                                            ytmp, o_ps[0:64, :], b_sb2,
                                            Alu.mult)
                                        nc.sync.dma_start(
                                            yt[64:128, ho, q0:q0 + 512], ytmp)

                                pending.append(rest)

                    flush_tails()
                    emit_outproj(3)

    nc.finalize()
    return nc


def _prep_shards(x, Wq, bq, Wk, bk, Wv, bv, Wo, bo):
    f = np.float32
    theta = 1.0 / (ROPE_BASE ** (np.arange(0, HD, 2, dtype=f) / HD))  # [32]
    pos = np.arange(1, T + 1, dtype=f)
    ang = pos[:, None] * theta[None, :]  # [T, 32]
    j = (np.arange(128) % HD) % 32
    cosT = np.ascontiguousarray(np.cos(ang).T[j, :], dtype=f)  # [128, T]
    sinT = np.ascontiguousarray(np.sin(ang).T[j, :], dtype=f)
    # rotate-half permutation (with sign): rot[p] = sgn(p) * q[p ^ 32]
    prm = np.zeros((128, 128), dtype=f)
    pp = np.arange(128)
    prm[pp, pp ^ 32] = np.where((pp % HD) < 32, -1.0, 1.0)
    permT = np.ascontiguousarray(prm.T)

    # additive causal mask for the diagonal 128-block: keep c >= p
    cc = np.arange(128)[None, :]
    triadd = np.where(cc >= pp[:, None], 0.0, -1e30).astype(f)
    triadd = np.ascontiguousarray(triadd)

    ident = np.eye(128, dtype=f)
    vones = np.ones((128, 16, HPC), dtype=f)

    def col128(b_):  # [512] -> [128, 4] (partition-major per 128-tile)
        return np.ascontiguousarray(np.asarray(b_, dtype=f).reshape(4, 128).T)

    in_maps = []
    for c in range(N_CORES):
        b, hg = c // 2, c % 2
        sl = slice(hg * 512, hg * 512 + 512)
        in_maps.append({
            "x": np.ascontiguousarray(x[b], dtype=f),
            "wq": np.ascontiguousarray(Wq[sl, :].T, dtype=f),
            "wk": np.ascontiguousarray(Wk[sl, :].T, dtype=f),
            "wv": np.ascontiguousarray(Wv[sl, :].T, dtype=f),
            "wo": np.ascontiguousarray(Wo[:, sl].T, dtype=f),
            "bq": col128(bq[sl]),
            "bk": col128(bk[sl]),
            "bv_bc": np.ascontiguousarray(
                np.tile(np.asarray(bv[sl], dtype=f)[None, :], (128, 1))),
            "cosT": cosT, "sinT": sinT, "ident": ident, "vones": vones,
            "permT": permT, "triadd": triadd,
        })
    return in_maps


def _run(inputs, trace=False):
    from concourse import bass_utils

    if "nc" not in _cache:
        _cache["nc"] = _build_bass()
    nc = _cache["nc"]
    in_maps = _prep_shards(**inputs)
    # The remote device occasionally reports a transient unrecoverable
    # state right after loading a fresh NEFF; a retry reliably clears it.
    last_exc = None
    for _ in range(3):
        try:
            res = bass_utils.run_bass_kernel_spmd(
                nc, in_maps, core_ids=list(range(N_CORES)), trace=trace)
            break
        except Exception as e:  # noqa: BLE001
            last_exc = e
            import time
            time.sleep(2.0)
    else:
        raise last_exc

    bo = np.asarray(inputs["bo"], dtype=np.float32)
    out = np.empty((B, T, D), dtype=np.float32)
    for b in range(B):
        out[b] = (res.results[2 * b]["yT"].T
                  + res.results[2 * b + 1]["yT"].T + bo)
    return out, res


def kernel(**inputs):
    out, _ = _run(inputs, trace=False)
    return out



# revision 3
# speedup vs baseline: 1.1688x; 1.1688x over previous
"""Causal self-attention (RoPE) Trainium2 kernel — bf16 edition.

Problem: B=4, T=2048, D=1024, H=16 heads (hd=64), fp32 I/O.
  q,k,v = x@W{q,k,v}.T + b;  rope(q), rope(k);  causal softmax attention;
  y = att_out @ Wo.T + bo.

Sharding (8 cores): data parallel over batch (4), tensor parallel over
heads (2 groups of 8 heads). Core c handles batch c//2, head-group c%2.
Each core computes its 8 heads end-to-end plus the partial out-projection;
the host sums the two head-group partials per batch and adds bo.

All matmul operands are bf16 (PSUM accumulation stays fp32); the host
pre-casts x/weights/tables to bf16 and pre-transposes x, so no on-device
transposes are needed. Layout is [dim, time]:
  - Q.T/K.T = W @ x.T; RoPE fused: the bias-add + cos/sin multiplies run as
    two scalar_tensor_tensor DVE ops (rotate-half via a +/-1 permutation
    matmul on PE, rotated bias folded in on the host); final add on GpSimd
  - V projected per 128-token strip from x.T (stationary) with ones column
    appended for the softmax denominator
  - S.T = K_h @ Q_h.T per head-PAIR (even head at partitions 0-63, odd at
    64-127 — the two K=64 matmuls run concurrently in the PE array); one
    exp per pair over [128, 1024] writing bf16, diagonal tiles sliced to
    the valid region with a static [128,128] 0/-1e30 triangle added to the
    S-PSUM before exp
  - O.T = [V_h | 1].T @ A.T accumulated over key tiles; ones column gives
    the denominator at psum row 64; AV matmuls trail the S matmuls by LAG
    key-tile pairs with deferred tails so the PE FIFO never drains
  - normalization via reciprocal_approx_fast (DVE) + bf16 cast (GpSimd) +
    K=1 broadcast matmul
  - out.T = Wo_c.T @ Y.T, emitted per finished query chunk, written
    transposed fp32; host transposes
"""

import sys

sys.path.insert(0, "/opt/trn_rl_repo")

import numpy as np

B, T, D, H = 4, 2048, 1024, 16
HD = 64
ROPE_BASE = 10000.0
N_CORES = 8
HPC = 8  # heads per core
LAG = 5  # AV matmul lag behind S matmul (key-tile pairs)

_cache = {}


def _build_bass():
    import concourse.mybir as mybir
    import concourse.tile as tile
    from concourse import bacc

    f32 = mybir.dt.float32
    bf = mybir.dt.bfloat16
    Alu = mybir.AluOpType
    Act = mybir.ActivationFunctionType

    nc = bacc.Bacc()

    # ---- DRAM I/O (per-core shards; same NEFF on all 8 cores) ----
    x_d = nc.dram_tensor("xT", [D, T], bf, kind="ExternalInput")
    wq_d = nc.dram_tensor("wq", [D, 512], bf, kind="ExternalInput")
    wk_d = nc.dram_tensor("wk", [D, 512], bf, kind="ExternalInput")
    wv_d = nc.dram_tensor("wv", [D, 512], bf, kind="ExternalInput")
    wo_d = nc.dram_tensor("wo", [512, D], bf, kind="ExternalInput")
    bq_d = nc.dram_tensor("bq", [128, 4], f32, kind="ExternalInput")
    bk_d = nc.dram_tensor("bk", [128, 4], f32, kind="ExternalInput")
    rbq_d = nc.dram_tensor("rbq", [128, 4], f32, kind="ExternalInput")
    rbk_d = nc.dram_tensor("rbk", [128, 4], f32, kind="ExternalInput")
    bv_d = nc.dram_tensor("bv_bc", [128, 512], f32, kind="ExternalInput")
    cos_d = nc.dram_tensor("cosT", [128, T], bf, kind="ExternalInput")
    sin_d = nc.dram_tensor("sinT", [128, T], bf, kind="ExternalInput")
    perm_d = nc.dram_tensor("permT", [128, 128], bf, kind="ExternalInput")
    tri_d = nc.dram_tensor("triadd", [128, 128], f32, kind="ExternalInput")
    yt_d = nc.dram_tensor("yT", [D, T], f32, kind="ExternalOutput")

    x_v = x_d[:, :].rearrange("(ko p) t -> p ko t", p=128)
    wq_v = wq_d[:, :].rearrange("(ko p) m -> p ko m", p=128)
    wk_v = wk_d[:, :].rearrange("(ko p) m -> p ko m", p=128)
    wv_v = wv_d[:, :].rearrange("(ko p) m -> p ko m", p=128)
    wo_v = wo_d[:, :].rearrange("(ko p) n -> p ko n", p=128)

    with tile.TileContext(nc) as tc:
        with (
            tc.tile_pool(name="singles", bufs=1) as singles,
            tc.tile_pool(name="big", bufs=1) as big,
        ):
            ones_hi = singles.tile([65, 64], bf)
            nc.vector.memset(ones_hi, 1.0)
            bq_sb = singles.tile([128, 4], f32, tag="bq")
            nc.sync.dma_start(bq_sb, bq_d[:, :])
            bk_sb = singles.tile([128, 4], f32, tag="bk")
            nc.sync.dma_start(bk_sb, bk_d[:, :])
            rbq_sb = singles.tile([128, 4], f32, tag="rbq")
            nc.sync.dma_start(rbq_sb, rbq_d[:, :])
            rbk_sb = singles.tile([128, 4], f32, tag="rbk")
            nc.sync.dma_start(rbk_sb, rbk_d[:, :])
            perm_sb = singles.tile([128, 128], bf, tag="perm")
            nc.sync.dma_start(perm_sb, perm_d[:, :])
            tri_sb = singles.tile([128, 128], f32, tag="tri")
            nc.sync.dma_start(tri_sb, tri_d[:, :])

            # persistent activations (bf16 so matmuls can consume them)
            xt = big.tile([128, 8, T], bf, tag="xt")
            qt = big.tile([128, 4, T], bf, tag="qt")
            kt = big.tile([128, 4, T], bf, tag="kt")
            v_sb = big.tile([128, 16, HPC, 65], bf, tag="v")
            nc.gpsimd.memset(v_sb[:, :, :, 64:65], 1.0)

            # ================= Phase A: Q.T/K.T (roped), V ==================
            with (
                tc.tile_pool(name="pa_sb", bufs=1) as pa,
                tc.tile_pool(name="wcol_p", bufs=2) as wcol_p,
                tc.tile_pool(name="qa_p", bufs=3) as qa_p,
                tc.tile_pool(name="t1_p", bufs=3) as t1_p,
                tc.tile_pool(name="t2_p", bufs=3) as t2_p,
                tc.tile_pool(name="qkps", bufs=3, space="PSUM") as qkps,
                tc.tile_pool(name="rotps", bufs=2, space="PSUM") as rotps,
                tc.tile_pool(name="auxps", bufs=2, space="PSUM") as auxps,
            ):
                # wv first: the V matmuls of strip 0 sit early in the PE
                # FIFO and must not wait behind the cos/sin table DMAs
                wv_sb = pa.tile([128, 8, 512], bf, tag="wv")
                nc.scalar.dma_start(wv_sb, wv_v)
                bv_sb = pa.tile([128, 512], f32, tag="bv")
                nc.scalar.dma_start(bv_sb, bv_d[:, :])
                cos_sb = pa.tile([128, T], bf, tag="cos")
                nc.scalar.dma_start(cos_sb, cos_d[:, :])
                sin_sb = pa.tile([128, T], bf, tag="sin")
                nc.scalar.dma_start(sin_sb, sin_d[:, :])
                # x.T arrives pre-transposed; stream it in 4 token chunks
                for cc in range(4):
                    nc.sync.dma_start(
                        xt[:, :, cc * 512:(cc + 1) * 512],
                        x_v[:, :, cc * 512:(cc + 1) * 512])

                # --- A1: V projection per 128-token strip ---
                for gt in range(16):
                    psV = auxps.tile([128, 512], f32, tag="aux",
                                     name=f"psV_{gt}")
                    for ko in range(8):
                        nc.tensor.matmul(
                            psV, lhsT=xt[:, ko, gt * 128:(gt + 1) * 128],
                            rhs=wv_sb[:, ko, :],
                            start=(ko == 0), stop=(ko == 7))
                    nc.vector.tensor_tensor(
                        v_sb[:, gt, :, 0:64],
                        psV.rearrange("p (h d) -> p h d", h=HPC),
                        bv_sb.rearrange("p (h d) -> p h d", h=HPC),
                        Alu.add)

                # --- A2: Q.T and K.T with fused RoPE ---
                for wview, bcol, rbcol, dest in (
                    (wq_v, bq_sb, rbq_sb, qt),
                    (wk_v, bk_sb, rbk_sb, kt),
                ):
                    for qo in range(4):
                        wcol = wcol_p.tile([128, 8, 128], bf, tag="wcol")
                        nc.sync.dma_start(
                            wcol, wview[:, :, qo * 128:(qo + 1) * 128])
                        for tp in range(2):
                            # two 512-chunks in flight so the rot matmul's
                            # ScalarE dependency hides under the other
                            # chunk's accumulation matmuls
                            ps_l, qa_l = [], []
                            for tcc in (2 * tp, 2 * tp + 1):
                                c0 = tcc * 512
                                psA = qkps.tile([128, 512], f32, tag="psA")
                                for ko in range(8):
                                    nc.tensor.matmul(
                                        psA, lhsT=wcol[:, ko, :],
                                        rhs=xt[:, ko, c0:c0 + 512],
                                        start=(ko == 0), stop=(ko == 7))
                                qa0 = qa_p.tile([128, 512], bf, tag="qa")
                                nc.scalar.copy(qa0, psA)
                                ps_l.append(psA)
                                qa_l.append(qa0)
                            rot_l = []
                            for i in range(2):
                                rps = rotps.tile([128, 512], f32, tag="rot")
                                nc.tensor.matmul(
                                    rps, lhsT=perm_sb, rhs=qa_l[i],
                                    start=True, stop=True)
                                rot_l.append(rps)
                            for i in range(2):
                                ta = (2 * tp + i) * 512
                                tmp1 = t1_p.tile([128, 512], bf, tag="tmp1")
                                nc.vector.scalar_tensor_tensor(
                                    tmp1, ps_l[i], bcol[:, qo:qo + 1],
                                    cos_sb[:, ta:ta + 512],
                                    Alu.add, Alu.mult)
                                tmp2 = t2_p.tile([128, 512], bf, tag="tmp2")
                                nc.vector.scalar_tensor_tensor(
                                    tmp2, rot_l[i], rbcol[:, qo:qo + 1],
                                    sin_sb[:, ta:ta + 512],
                                    Alu.add, Alu.mult)
                                nc.gpsimd.tensor_tensor(
                                    dest[:, qo, ta:ta + 512], tmp1, tmp2,
                                    Alu.add)

            # ================= Phase B: attention ==========================
            with tc.tile_pool(name="pb_keep", bufs=1) as pb_keep:
                yt = pb_keep.tile([128, 4, T], bf, tag="yt")
                wo_sb = pb_keep.tile([128, 4, D], bf, tag="wo")
                nc.scalar.dma_start(wo_sb, wo_v)
                with (
                    tc.tile_pool(name="at_p", bufs=7) as at_p,
                    tc.tile_pool(name="rec_p", bufs=4) as rec_p,
                    tc.tile_pool(name="ytmp_p", bufs=2) as ytmp_p,
                    tc.tile_pool(name="orow_p", bufs=4) as orow_p,
                    tc.tile_pool(name="sps", bufs=2, space="PSUM") as sps,
                    tc.tile_pool(name="ops", bufs=4, space="PSUM") as ops,
                ):
                    def emit_outproj(cj, dos=range(8)):
                        p0 = cj * 512
                        for do in dos:
                            ps2 = sps.tile([128, 1024], f32, tag="sps",
                                           name=f"op_{cj}_{do}")
                            ps = ps2[:, 0:512]
                            for ko in range(4):
                                nc.tensor.matmul(
                                    ps, lhsT=wo_sb[:, ko,
                                                   do * 128:(do + 1) * 128],
                                    rhs=yt[:, ko, p0:p0 + 512],
                                    start=(ko == 0), stop=(ko == 3))
                            orow = orow_p.tile([128, 512], f32, tag="orow")
                            nc.vector.tensor_copy(orow, ps)
                            nc.sync.dma_start(
                                yt_d[do * 128:(do + 1) * 128, p0:p0 + 512],
                                orow)

                    pending = []
                    pending_avs = []
                    pending_recips = []

                    def flush_avs():
                        for fn in pending_avs:
                            fn()
                        pending_avs.clear()
                        for fn in pending_recips:
                            fn()
                        pending_recips.clear()

                    def flush_tails():
                        flush_avs()
                        for fn in pending:
                            fn()
                        pending.clear()

                    for ci in range(4):
                        q0 = ci * 512
                        nkt = 4 * ci + 4
                        for ho in range(4):
                            if ho == 1 and ci > 0:
                                flush_tails()
                                emit_outproj(ci - 1, range(0, 4))
                            elif ho == 2 and ci > 0:
                                emit_outproj(ci - 1, range(4, 8))
                            o_pair = [
                                ops.tile([65, 512], f32, tag="ops",
                                         name=f"ops_{ci}_{ho}_{g_}")
                                for g_ in range(2)]
                            at2s = {}

                            def do_av(k_i, o_pair=o_pair, at2s=at2s, nkt=nkt,
                                      ho=ho, ci=ci):
                                at2 = at2s.pop(k_i)
                                sdx = k_i - 4 * ci
                                f0 = max(0, sdx) * 128
                                for g in range(2):
                                    nc.tensor.matmul(
                                        o_pair[g][:, f0:512],
                                        lhsT=v_sb[:, k_i, 2 * ho + g, :],
                                        rhs=at2[:, g * 512 + f0:
                                                (g + 1) * 512],
                                        start=(k_i == 0),
                                        stop=(k_i == nkt - 1))

                            for kt_i in range(nkt):
                                sdx = kt_i - 4 * ci
                                f0 = max(0, sdx) * 128
                                s_ps2 = sps.tile([128, 1024], f32, tag="sps")
                                for g in range(2):
                                    hp = g * 64
                                    nc.tensor.matmul(
                                        s_ps2[:, g * 512 + f0:(g + 1) * 512],
                                        lhsT=kt[hp:hp + 64, ho,
                                                kt_i * 128:(kt_i + 1) * 128],
                                        rhs=qt[hp:hp + 64, ho,
                                               q0 + f0:q0 + 512],
                                        start=True, stop=True)
                                at2 = at_p.tile([128, 1024], bf, tag="at")
                                av = at2.rearrange("p (g q) -> p g q", g=2)
                                sv = s_ps2.rearrange("p (g q) -> p g q", g=2)
                                if sdx >= 0:
                                    nc.vector.tensor_tensor(
                                        sv[:, :, f0:f0 + 128],
                                        sv[:, :, f0:f0 + 128],
                                        tri_sb[:, None, :].to_broadcast(
                                            (128, 2, 128)),
                                        Alu.add)
                                nc.scalar.activation(
                                    av[:, :, f0:512], sv[:, :, f0:512],
                                    Act.Exp, scale=0.125)
                                at2s[kt_i] = at2
                                if kt_i == 0:
                                    flush_avs()
                                elif kt_i == 1:
                                    flush_tails()
                                if kt_i >= LAG:
                                    do_av(kt_i - LAG)
                            for k_i in range(max(0, nkt - LAG), nkt):
                                pending_avs.append(
                                    lambda k_i=k_i, do_av=do_av: do_av(k_i))

                            recs = [rec_p.tile([65, 512], f32, tag="rec",
                                               name=f"rec_{ci}_{ho}_{g_}")
                                    for g_ in range(2)]
                            rbfs = [rec_p.tile([65, 512], bf, tag="recb",
                                               name=f"recb_{ci}_{ho}_{g_}")
                                    for g_ in range(2)]
                            for g in range(2):
                                o_ps = o_pair[g]
                                rec = recs[g]
                                rbf = rbfs[g]

                                def recip(o_ps=o_ps, rec=rec, rbf=rbf):
                                    nc.vector.reciprocal(
                                        rec[64:65, :], o_ps[64:65, :])
                                    nc.gpsimd.tensor_copy(
                                        rbf[64:65, :], rec[64:65, :])

                                pending_recips.append(recip)

                                def rest(g=g, o_ps=o_ps, rbf=rbf, ho=ho,
                                         q0=q0, ci=ci):
                                    b_ps = ops.tile(
                                        [64, 512], f32, tag="ops",
                                        name=f"bps_{ci}_{ho}_{g}")
                                    nc.tensor.matmul(
                                        b_ps, lhsT=ones_hi[64:65, :],
                                        rhs=rbf[64:65, :],
                                        start=True, stop=True)
                                    b_sb2 = rec_p.tile(
                                        [64, 512], bf, tag="bsb")
                                    nc.vector.tensor_copy(b_sb2, b_ps)
                                    if g == 0:
                                        nc.vector.tensor_tensor(
                                            yt[0:64, ho, q0:q0 + 512],
                                            o_ps[0:64, :], b_sb2, Alu.mult)
                                    else:
                                        ytmp = ytmp_p.tile(
                                            [64, 512], bf, tag="ytmp")
                                        nc.vector.tensor_tensor(
                                            ytmp, o_ps[0:64, :], b_sb2,
                                            Alu.mult)
                                        nc.sync.dma_start(
                                            yt[64:128, ho, q0:q0 + 512], ytmp)

                                pending.append(rest)

                    flush_tails()
                    emit_outproj(3)

    nc.finalize()
    return nc


def _prep_shards(x, Wq, bq, Wk, bk, Wv, bv, Wo, bo):
    import ml_dtypes

    f = np.float32
    bft = ml_dtypes.bfloat16
    theta = 1.0 / (ROPE_BASE ** (np.arange(0, HD, 2, dtype=f) / HD))  # [32]
    pos = np.arange(1, T + 1, dtype=f)
    ang = pos[:, None] * theta[None, :]  # [T, 32]
    j = (np.arange(128) % HD) % 32
    cosT = np.ascontiguousarray(np.cos(ang).T[j, :].astype(bft))  # [128, T]
    sinT = np.ascontiguousarray(np.sin(ang).T[j, :].astype(bft))
    # rotate-half permutation (with sign): rot[p] = sgn(p) * q[p ^ 32]
    prm = np.zeros((128, 128), dtype=f)
    pp = np.arange(128)
    prm[pp, pp ^ 32] = np.where((pp % HD) < 32, -1.0, 1.0)
    permT = np.ascontiguousarray(prm.T.astype(bft))

    # additive causal mask for the diagonal 128-block: keep c >= p
    cc = np.arange(128)[None, :]
    triadd = np.where(cc >= pp[:, None], 0.0, -1e30).astype(f)
    triadd = np.ascontiguousarray(triadd)

    def col128(b_):  # [512] -> [128, 4] (partition-major per 128-tile)
        return np.ascontiguousarray(np.asarray(b_, dtype=f).reshape(4, 128).T)

    in_maps = []
    for c in range(N_CORES):
        b, hg = c // 2, c % 2
        sl = slice(hg * 512, hg * 512 + 512)
        bqc, bkc = col128(bq[sl]), col128(bk[sl])
        in_maps.append({
            "xT": np.ascontiguousarray(np.asarray(x[b], dtype=f).T
                                       .astype(bft)),
            "wq": np.ascontiguousarray(np.asarray(Wq[sl, :], dtype=f).T
                                       .astype(bft)),
            "wk": np.ascontiguousarray(np.asarray(Wk[sl, :], dtype=f).T
                                       .astype(bft)),
            "wv": np.ascontiguousarray(np.asarray(Wv[sl, :], dtype=f).T
                                       .astype(bft)),
            "wo": np.ascontiguousarray(np.asarray(Wo[:, sl], dtype=f).T
                                       .astype(bft)),
            "bq": bqc, "bk": bkc,
            "rbq": np.ascontiguousarray(prm @ bqc),
            "rbk": np.ascontiguousarray(prm @ bkc),
            "bv_bc": np.ascontiguousarray(
                np.tile(np.asarray(bv[sl], dtype=f)[None, :], (128, 1))),
            "cosT": cosT, "sinT": sinT,
            "permT": permT, "triadd": triadd,
        })
    return in_maps


def _run(inputs, trace=False):
    from concourse import bass_utils

    if "nc" not in _cache:
        _cache["nc"] = _build_bass()
    nc = _cache["nc"]
    in_maps = _prep_shards(**inputs)
    # The remote device occasionally reports a transient unrecoverable
    # state right after loading a fresh NEFF; a retry reliably clears it.
    last_exc = None
    for _ in range(3):
        try:
            res = bass_utils.run_bass_kernel_spmd(
                nc, in_maps, core_ids=list(range(N_CORES)), trace=trace)
            break
        except Exception as e:  # noqa: BLE001
            last_exc = e
            import time
            time.sleep(2.0)
    else:
        raise last_exc

    bo = np.asarray(inputs["bo"], dtype=np.float32)
    out = np.empty((B, T, D), dtype=np.float32)
    for b in range(B):
        out[b] = (res.results[2 * b]["yT"].T
                  + res.results[2 * b + 1]["yT"].T + bo)
    return out, res


def kernel(**inputs):
    out, _ = _run(inputs, trace=False)
    return out


# revision 10
# speedup vs baseline: 1.4306x; 1.2239x over previous
"""Causal self-attention (RoPE) Trainium2 kernel — bf16 edition.

Problem: B=4, T=2048, D=1024, H=16 heads (hd=64), fp32 I/O.
  q,k,v = x@W{q,k,v}.T + b;  rope(q), rope(k);  causal softmax attention;
  y = att_out @ Wo.T + bo.

Sharding (8 cores): data parallel over batch (4), tensor parallel over
heads (2 groups of 8 heads). Core c handles batch c//2, head-group c%2.
Each core computes its 8 heads end-to-end plus the partial out-projection;
the host sums the two head-group partials per batch and adds bo.

All matmul operands are bf16 (PSUM accumulation stays fp32); the host
pre-casts x/weights/tables to bf16 and pre-transposes x, so no on-device
transposes are needed. Layout is [dim, time]:
  - Q.T/K.T = W @ x.T; RoPE fused: the bias-add + cos/sin multiplies run as
    two scalar_tensor_tensor DVE ops (rotate-half via a +/-1 permutation
    matmul on PE, rotated bias folded in on the host); final add on GpSimd
  - V projected per 128-token strip from x.T (stationary) with ones column
    appended for the softmax denominator
  - S.T = K_h @ Q_h.T per head-PAIR (even head at partitions 0-63, odd at
    64-127 — the two K=64 matmuls run concurrently in the PE array); one
    exp per pair over [128, 1024] writing bf16, diagonal tiles sliced to
    the valid region with a static [128,128] 0/-1e30 triangle added to the
    S-PSUM before exp
  - O.T = [V_h | 1].T @ A.T accumulated over key tiles; ones column gives
    the denominator at psum row 64; AV matmuls trail the S matmuls by LAG
    key-tile pairs with deferred tails so the PE FIFO never drains
  - normalization via reciprocal_approx_fast (DVE) + bf16 cast (GpSimd) +
    K=1 broadcast matmul
  - out.T = Wo_c.T @ Y.T, emitted per finished query chunk, written
    transposed fp32; host transposes
"""

import sys

sys.path.insert(0, "/opt/trn_rl_repo")

import numpy as np

B, T, D, H = 4, 2048, 1024, 16
HD = 64
ROPE_BASE = 10000.0
N_CORES = 8
HPC = 8  # heads per core
LAG = 5  # AV matmul lag behind S matmul (key-tile pairs)

_cache = {}


def _build_bass():
    import concourse.mybir as mybir
    import concourse.tile as tile
    from concourse import bacc

    f32 = mybir.dt.float32
    bf = mybir.dt.bfloat16
    Alu = mybir.AluOpType
    Act = mybir.ActivationFunctionType

    nc = bacc.Bacc()

    # ---- DRAM I/O (per-core shards; same NEFF on all 8 cores) ----
    x_d = nc.dram_tensor("xT", [D, T], bf, kind="ExternalInput")
    wq_d = nc.dram_tensor("wq", [D, 512], bf, kind="ExternalInput")
    wk_d = nc.dram_tensor("wk", [D, 512], bf, kind="ExternalInput")
    wv_d = nc.dram_tensor("wv", [D, 512], bf, kind="ExternalInput")
    wo_d = nc.dram_tensor("wo", [512, D], bf, kind="ExternalInput")
    bq_d = nc.dram_tensor("bq", [128, 4], f32, kind="ExternalInput")
    bk_d = nc.dram_tensor("bk", [128, 4], f32, kind="ExternalInput")
    rbq_d = nc.dram_tensor("rbq", [128, 4], f32, kind="ExternalInput")
    rbk_d = nc.dram_tensor("rbk", [128, 4], f32, kind="ExternalInput")
    bv_d = nc.dram_tensor("bv_bc", [128, 512], f32, kind="ExternalInput")
    cos_d = nc.dram_tensor("cosT", [128, T], bf, kind="ExternalInput")
    sin_d = nc.dram_tensor("sinT", [128, T], bf, kind="ExternalInput")
    perm_d = nc.dram_tensor("permT", [128, 128], bf, kind="ExternalInput")
    tri_d = nc.dram_tensor("triadd", [128, 128], f32, kind="ExternalInput")
    yt_d = nc.dram_tensor("yT", [D, T], f32, kind="ExternalOutput")

    x_v = x_d[:, :].rearrange("(ko p) t -> p ko t", p=128)
    wq_v = wq_d[:, :].rearrange("(ko p) m -> p ko m", p=128)
    wk_v = wk_d[:, :].rearrange("(ko p) m -> p ko m", p=128)
    wv_v = wv_d[:, :].rearrange("(ko p) m -> p ko m", p=128)
    wo_v = wo_d[:, :].rearrange("(ko p) n -> p ko n", p=128)

    with tile.TileContext(nc) as tc:
        with (
            tc.tile_pool(name="singles", bufs=1) as singles,
            tc.tile_pool(name="big", bufs=1) as big,
        ):
            ones1 = singles.tile([1, 64], bf)
            nc.vector.memset(ones1, 1.0)
            bq_sb = singles.tile([128, 4], f32, tag="bq")
            nc.sync.dma_start(bq_sb, bq_d[:, :])
            bk_sb = singles.tile([128, 4], f32, tag="bk")
            nc.sync.dma_start(bk_sb, bk_d[:, :])
            rbq_sb = singles.tile([128, 4], f32, tag="rbq")
            nc.sync.dma_start(rbq_sb, rbq_d[:, :])
            rbk_sb = singles.tile([128, 4], f32, tag="rbk")
            nc.sync.dma_start(rbk_sb, rbk_d[:, :])
            perm_sb = singles.tile([128, 128], bf, tag="perm")
            nc.sync.dma_start(perm_sb, perm_d[:, :])
            tri_sb = singles.tile([128, 128], f32, tag="tri")
            nc.sync.dma_start(tri_sb, tri_d[:, :])

            # persistent activations (bf16 so matmuls can consume them)
            xt = big.tile([128, 8, T], bf, tag="xt")
            qt = big.tile([128, 4, T], bf, tag="qt")
            kt = big.tile([128, 4, T], bf, tag="kt")
            v_sb = big.tile([128, 16, HPC, 65], bf, tag="v")
            nc.gpsimd.memset(v_sb[:, :, :, 64:65], 1.0)

            # ================= Phase A: Q.T/K.T (roped), V ==================
            with (
                tc.tile_pool(name="pa_sb", bufs=1) as pa,
                tc.tile_pool(name="wcol_p", bufs=2) as wcol_p,
                tc.tile_pool(name="qa_p", bufs=3) as qa_p,
                tc.tile_pool(name="t1_p", bufs=3) as t1_p,
                tc.tile_pool(name="t2_p", bufs=3) as t2_p,
                tc.tile_pool(name="qkps", bufs=3, space="PSUM") as qkps,
                tc.tile_pool(name="rotps", bufs=2, space="PSUM") as rotps,
                tc.tile_pool(name="auxps", bufs=2, space="PSUM") as auxps,
            ):
                # wv first: the V matmuls of strip 0 sit early in the PE
                # FIFO and must not wait behind the cos/sin table DMAs
                wv_sb = pa.tile([128, 8, 512], bf, tag="wv")
                nc.scalar.dma_start(wv_sb, wv_v)
                bv_sb = pa.tile([128, 512], f32, tag="bv")
                nc.scalar.dma_start(bv_sb, bv_d[:, :])
                cos_sb = pa.tile([128, T], bf, tag="cos")
                nc.scalar.dma_start(cos_sb, cos_d[:, :])
                sin_sb = pa.tile([128, T], bf, tag="sin")
                nc.scalar.dma_start(sin_sb, sin_d[:, :])
                # x.T arrives pre-transposed; stream it in 4 token chunks
                for cc in range(4):
                    nc.sync.dma_start(
                        xt[:, :, cc * 512:(cc + 1) * 512],
                        x_v[:, :, cc * 512:(cc + 1) * 512])

                # --- A1: V projection per 128-token strip ---
                for gt in range(16):
                    psV = auxps.tile([128, 512], f32, tag="aux",
                                     name=f"psV_{gt}")
                    for ko in range(8):
                        nc.tensor.matmul(
                            psV, lhsT=xt[:, ko, gt * 128:(gt + 1) * 128],
                            rhs=wv_sb[:, ko, :],
                            start=(ko == 0), stop=(ko == 7))
                    nc.vector.tensor_tensor(
                        v_sb[:, gt, :, 0:64],
                        psV.rearrange("p (h d) -> p h d", h=HPC),
                        bv_sb.rearrange("p (h d) -> p h d", h=HPC),
                        Alu.add)

                # --- A2: Q.T and K.T with fused RoPE ---
                for wview, bcol, rbcol, dest in (
                    (wq_v, bq_sb, rbq_sb, qt),
                    (wk_v, bk_sb, rbk_sb, kt),
                ):
                    for qo in range(4):
                        wcol = wcol_p.tile([128, 8, 128], bf, tag="wcol")
                        nc.sync.dma_start(
                            wcol, wview[:, :, qo * 128:(qo + 1) * 128])
                        for tp in range(2):
                            # two 512-chunks in flight so the rot matmul's
                            # ScalarE dependency hides under the other
                            # chunk's accumulation matmuls
                            ps_l, qa_l = [], []
                            for tcc in (2 * tp, 2 * tp + 1):
                                c0 = tcc * 512
                                psA = qkps.tile([128, 512], f32, tag="psA")
                                for ko in range(8):
                                    nc.tensor.matmul(
                                        psA, lhsT=wcol[:, ko, :],
                                        rhs=xt[:, ko, c0:c0 + 512],
                                        start=(ko == 0), stop=(ko == 7))
                                qa0 = qa_p.tile([128, 512], bf, tag="qa")
                                nc.scalar.copy(qa0, psA)
                                ps_l.append(psA)
                                qa_l.append(qa0)
                            rot_l = []
                            for i in range(2):
                                rps = rotps.tile([128, 512], f32, tag="rot")
                                nc.tensor.matmul(
                                    rps, lhsT=perm_sb, rhs=qa_l[i],
                                    start=True, stop=True)
                                rot_l.append(rps)
                            for i in range(2):
                                ta = (2 * tp + i) * 512
                                tmp1 = t1_p.tile([128, 512], bf, tag="tmp1")
                                nc.vector.scalar_tensor_tensor(
                                    tmp1, ps_l[i], bcol[:, qo:qo + 1],
                                    cos_sb[:, ta:ta + 512],
                                    Alu.add, Alu.mult)
                                tmp2 = t2_p.tile([128, 512], bf, tag="tmp2")
                                nc.vector.scalar_tensor_tensor(
                                    tmp2, rot_l[i], rbcol[:, qo:qo + 1],
                                    sin_sb[:, ta:ta + 512],
                                    Alu.add, Alu.mult)
                                nc.gpsimd.tensor_tensor(
                                    dest[:, qo, ta:ta + 512], tmp1, tmp2,
                                    Alu.add)

            # ================= Phase B: attention ==========================
            with tc.tile_pool(name="pb_keep", bufs=1) as pb_keep:
                yt = pb_keep.tile([128, 4, T], bf, tag="yt")
                wo_sb = pb_keep.tile([128, 4, D], bf, tag="wo")
                nc.scalar.dma_start(wo_sb, wo_v)
                with (
                    tc.tile_pool(name="at_p", bufs=7) as at_p,
                    tc.tile_pool(name="sp_p", bufs=4) as sp_p,
                    tc.tile_pool(name="spb_p", bufs=4) as spb_p,
                    tc.tile_pool(name="rbf_p", bufs=4) as rbf_p,
                    tc.tile_pool(name="osb_p", bufs=4) as osb_p,
                    tc.tile_pool(name="bsb_p", bufs=4) as bsb_p,
                    tc.tile_pool(name="ytmp_p", bufs=2) as ytmp_p,
                    tc.tile_pool(name="orow_p", bufs=4) as orow_p,
                    tc.tile_pool(name="sps", bufs=2, space="PSUM") as sps,
                    tc.tile_pool(name="ops", bufs=3, space="PSUM") as ops,
                ):
                    def emit_outproj(cj, dos=range(8)):
                        p0 = cj * 512
                        for do in dos:
                            ps2 = sps.tile([128, 1024], f32, tag="sps",
                                           name=f"op_{cj}_{do}")
                            ps = ps2[:, 0:512]
                            for ko in range(4):
                                nc.tensor.matmul(
                                    ps, lhsT=wo_sb[:, ko,
                                                   do * 128:(do + 1) * 128],
                                    rhs=yt[:, ko, p0:p0 + 512],
                                    start=(ko == 0), stop=(ko == 3))
                            orow = orow_p.tile([128, 512], f32, tag="orow")
                            nc.vector.tensor_copy(orow, ps)
                            nc.sync.dma_start(
                                yt_d[do * 128:(do + 1) * 128, p0:p0 + 512],
                                orow)

                    pending = []
                    pending_avs = []
                    pending_recips = []

                    def flush_avs():
                        for fn in pending_avs:
                            fn()
                        pending_avs.clear()
                        for fn in pending_recips:
                            fn()
                        pending_recips.clear()

                    def flush_tails():
                        flush_avs()
                        for fn in pending:
                            fn()
                        pending.clear()

                    for ci in range(4):
                        q0 = ci * 512
                        nkt = 4 * ci + 4
                        for ho in range(4):
                            if ho == 1 and ci > 0:
                                flush_tails()
                                emit_outproj(ci - 1, range(0, 4))
                            elif ho == 2 and ci > 0:
                                emit_outproj(ci - 1, range(4, 8))
                            o_pair = [
                                ops.tile([65, 512], f32, tag="ops",
                                         name=f"ops_{ci}_{ho}_{g_}")
                                for g_ in range(2)]
                            at2s = {}

                            def do_av(k_i, o_pair=o_pair, at2s=at2s, nkt=nkt,
                                      ho=ho, ci=ci):
                                at2 = at2s.pop(k_i)
                                sdx = k_i - 4 * ci
                                f0 = max(0, sdx) * 128
                                for g in range(2):
                                    nc.tensor.matmul(
                                        o_pair[g][:, f0:512],
                                        lhsT=v_sb[:, k_i, 2 * ho + g, :],
                                        rhs=at2[:, g * 512 + f0:
                                                (g + 1) * 512],
                                        start=(k_i == 0),
                                        stop=(k_i == nkt - 1))

                            for kt_i in range(nkt):
                                sdx = kt_i - 4 * ci
                                f0 = max(0, sdx) * 128
                                s_ps2 = sps.tile([128, 1024], f32, tag="sps")
                                for g in range(2):
                                    hp = g * 64
                                    nc.tensor.matmul(
                                        s_ps2[:, g * 512 + f0:(g + 1) * 512],
                                        lhsT=kt[hp:hp + 64, ho,
                                                kt_i * 128:(kt_i + 1) * 128],
                                        rhs=qt[hp:hp + 64, ho,
                                               q0 + f0:q0 + 512],
                                        start=True, stop=True)
                                at2 = at_p.tile([128, 1024], bf, tag="at")
                                av = at2.rearrange("p (g q) -> p g q", g=2)
                                sv = s_ps2.rearrange("p (g q) -> p g q", g=2)
                                if sdx >= 0:
                                    nc.vector.tensor_tensor(
                                        sv[:, :, f0:f0 + 128],
                                        sv[:, :, f0:f0 + 128],
                                        tri_sb[:, None, :].to_broadcast(
                                            (128, 2, 128)),
                                        Alu.add)
                                nc.scalar.activation(
                                    av[:, :, f0:512], sv[:, :, f0:512],
                                    Act.Exp, scale=0.125)
                                at2s[kt_i] = at2
                                if kt_i == 0:
                                    flush_avs()
                                elif kt_i == 1:
                                    flush_tails()
                                if kt_i >= LAG:
                                    do_av(kt_i - LAG)
                            for k_i in range(max(0, nkt - LAG), nkt):
                                pending_avs.append(
                                    lambda k_i=k_i, do_av=do_av: do_av(k_i))

                            for g in range(2):
                                o_ps = o_pair[g]
                                sp = sp_p.tile([128, 4], bf, tag="sp",
                                               name=f"sp_{ci}_{ho}_{g}")
                                o_sb = osb_p.tile([65, 512], bf, tag="osb",
                                                  name=f"osb_{ci}_{ho}_{g}")

                                def recip(o_ps=o_ps, sp=sp, o_sb=o_sb):
                                    # numerator+denominator out of PSUM in
                                    # one copy so the bank frees
                                    # immediately; then spread the
                                    # denominator row over 128 partitions
                                    nc.vector.tensor_copy(o_sb, o_ps)
                                    nc.sync.dma_start(sp, o_sb[64:65, :])

                                pending_recips.append(recip)

                                def rest(g=g, sp=sp, o_sb=o_sb, ho=ho,
                                         q0=q0, ci=ci):
                                    rsp = sp_p.tile([128, 4], f32, tag="rsp")
                                    nc.vector.reciprocal(rsp, sp)
                                    rsb = spb_p.tile([128, 4], bf, tag="rsb")
                                    nc.vector.tensor_copy(rsb, rsp)
                                    rbf = rbf_p.tile([1, 512], bf, tag="rbf")
                                    nc.sync.dma_start(rbf, rsb)
                                    b_ps = ops.tile(
                                        [64, 512], f32, tag="ops",
                                        name=f"bps_{ci}_{ho}_{g}")
                                    nc.tensor.matmul(
                                        b_ps, lhsT=ones1, rhs=rbf,
                                        start=True, stop=True)
                                    if g == 0:
                                        nc.vector.tensor_tensor(
                                            yt[0:64, ho, q0:q0 + 512],
                                            o_sb[0:64, :], b_ps, Alu.mult)
                                    else:
                                        ytmp = ytmp_p.tile(
                                            [64, 512], bf, tag="ytmp")
                                        nc.vector.tensor_tensor(
                                            ytmp, o_sb[0:64, :], b_ps,
                                            Alu.mult)
                                        nc.sync.dma_start(
                                            yt[64:128, ho, q0:q0 + 512], ytmp)

                                pending.append(rest)

                    flush_tails()
                    emit_outproj(3)

    nc.finalize()
    return nc


def _prep_shards(x, Wq, bq, Wk, bk, Wv, bv, Wo, bo):
    import ml_dtypes

    f = np.float32
    bft = ml_dtypes.bfloat16
    theta = 1.0 / (ROPE_BASE ** (np.arange(0, HD, 2, dtype=f) / HD))  # [32]
    pos = np.arange(1, T + 1, dtype=f)
    ang = pos[:, None] * theta[None, :]  # [T, 32]
    j = (np.arange(128) % HD) % 32
    cosT = np.ascontiguousarray(np.cos(ang).T[j, :].astype(bft))  # [128, T]
    sinT = np.ascontiguousarray(np.sin(ang).T[j, :].astype(bft))
    # rotate-half permutation (with sign): rot[p] = sgn(p) * q[p ^ 32]
    prm = np.zeros((128, 128), dtype=f)
    pp = np.arange(128)
    prm[pp, pp ^ 32] = np.where((pp % HD) < 32, -1.0, 1.0)
    permT = np.ascontiguousarray(prm.T.astype(bft))

    # additive causal mask for the diagonal 128-block: keep c >= p
    cc = np.arange(128)[None, :]
    triadd = np.where(cc >= pp[:, None], 0.0, -1e30).astype(f)
    triadd = np.ascontiguousarray(triadd)

    def col128(b_):  # [512] -> [128, 4] (partition-major per 128-tile)
        return np.ascontiguousarray(np.asarray(b_, dtype=f).reshape(4, 128).T)

    in_maps = []
    for c in range(N_CORES):
        b, hg = c // 2, c % 2
        sl = slice(hg * 512, hg * 512 + 512)
        bqc, bkc = col128(bq[sl]), col128(bk[sl])
        in_maps.append({
            "xT": np.ascontiguousarray(np.asarray(x[b], dtype=f).T
                                       .astype(bft)),
            "wq": np.ascontiguousarray(np.asarray(Wq[sl, :], dtype=f).T
                                       .astype(bft)),
            "wk": np.ascontiguousarray(np.asarray(Wk[sl, :], dtype=f).T
                                       .astype(bft)),
            "wv": np.ascontiguousarray(np.asarray(Wv[sl, :], dtype=f).T
                                       .astype(bft)),
            "wo": np.ascontiguousarray(np.asarray(Wo[:, sl], dtype=f).T
                                       .astype(bft)),
            "bq": bqc, "bk": bkc,
            "rbq": np.ascontiguousarray(prm @ bqc),
            "rbk": np.ascontiguousarray(prm @ bkc),
            "bv_bc": np.ascontiguousarray(
                np.tile(np.asarray(bv[sl], dtype=f)[None, :], (128, 1))),
            "cosT": cosT, "sinT": sinT,
            "permT": permT, "triadd": triadd,
        })
    return in_maps


def _run(inputs, trace=False):
    from concourse import bass_utils

    if "nc" not in _cache:
        _cache["nc"] = _build_bass()
    nc = _cache["nc"]
    in_maps = _prep_shards(**inputs)
    # The remote device occasionally reports a transient unrecoverable
    # state right after loading a fresh NEFF; a retry reliably clears it.
    last_exc = None
    for _ in range(3):
        try:
            res = bass_utils.run_bass_kernel_spmd(
                nc, in_maps, core_ids=list(range(N_CORES)), trace=trace)
            break
        except Exception as e:  # noqa: BLE001
            last_exc = e
            import time
            time.sleep(2.0)
    else:
        raise last_exc

    bo = np.asarray(inputs["bo"], dtype=np.float32)
    out = np.empty((B, T, D), dtype=np.float32)
    for b in range(B):
        out[b] = (res.results[2 * b]["yT"].T
                  + res.results[2 * b + 1]["yT"].T + bo)
    return out, res


def kernel(**inputs):
    out, _ = _run(inputs, trace=False)
    return out


# revision 13
# speedup vs baseline: 1.6947x; 1.1847x over previous
"""Causal self-attention (RoPE) Trainium2 kernel — bf16, software-pipelined.

Problem: B=4, T=2048, D=1024, H=16 heads (hd=64), fp32 I/O.
  q,k,v = x@W{q,k,v}.T + b;  rope(q), rope(k);  causal softmax attention;
  y = att_out @ Wo.T + bo.

Sharding (8 cores): data parallel over batch (4), tensor parallel over
heads (2 groups of 8 heads). Core c handles batch c//2, head-group c%2.
Each core computes its 8 heads end-to-end plus the partial out-projection;
the host sums the two head-group partials per batch and adds bo.

All matmul operands are bf16 (PSUM accumulates fp32); the host pre-casts
x/weights/tables to bf16 and pre-transposes x. Layout is [dim, time].

The kernel is one software-pipelined stream: the attention of query chunk
ci is ScalarE(exp)-paced, so the projection jobs for chunk ci+1 and the
out-projection of chunk ci-1 are injected between its key tiles to keep
the PE (and its HAM clock gate) busy:
  - Q.T/K.T = W @ x.T per (dest, head-pair, chunk) job; RoPE fused: ScalarE
    makes a bf16 copy of the PSUM for the rotate-half permutation matmul
    (PE), two scalar_tensor_tensor DVE ops apply (psum+bias)*cos and
    (rot+rot_bias)*sin (rotated bias precomputed on host), GpSimd adds
  - V per 128-token strip with a ones column for the softmax denominator
  - S.T = K_h @ Q_h.T per head-PAIR (even head at partitions 0-63, odd at
    64-127; the two K=64 matmuls run concurrently in the PE array); one exp
    per pair over [128, 1024] (bf16 out), diagonal tiles sliced with a
    static 0/-1e30 triangle added to the S-PSUM first
  - O.T = [V_h | 1].T @ A.T per key tile; AVs trail S by LAG tiles with
    deferred tails; O+denominator leave PSUM in one bf16 copy immediately
    (frees the bank), the denominator row is spread over 16 partitions by
    DMA so the exact DVE reciprocal is cheap, then cast, DMA'd back to one
    row, partition-broadcast via a K=1 matmul, and multiplied in
  - out.T = Wo_c.T @ Y.T written transposed fp32; host transposes
"""

import sys

sys.path.insert(0, "/opt/trn_rl_repo")

import numpy as np

B, T, D, H = 4, 2048, 1024, 16
HD = 64
ROPE_BASE = 10000.0
N_CORES = 8
HPC = 8  # heads per core
LAG = 5  # AV matmul lag behind S matmul (key tiles)

_cache = {}


def _build_bass():
    import concourse.mybir as mybir
    import concourse.tile as tile
    from concourse import bacc

    f32 = mybir.dt.float32
    bf = mybir.dt.bfloat16
    Alu = mybir.AluOpType
    Act = mybir.ActivationFunctionType

    nc = bacc.Bacc()

    # ---- DRAM I/O (per-core shards; same NEFF on all 8 cores) ----
    x_d = nc.dram_tensor("xT", [D, T], bf, kind="ExternalInput")
    wq_d = nc.dram_tensor("wq", [D, 512], bf, kind="ExternalInput")
    wk_d = nc.dram_tensor("wk", [D, 512], bf, kind="ExternalInput")
    wv_d = nc.dram_tensor("wv", [D, 512], bf, kind="ExternalInput")
    wo_d = nc.dram_tensor("wo", [512, D], bf, kind="ExternalInput")
    bq_d = nc.dram_tensor("bq", [128, 4], f32, kind="ExternalInput")
    bk_d = nc.dram_tensor("bk", [128, 4], f32, kind="ExternalInput")
    rbq_d = nc.dram_tensor("rbq", [128, 4], f32, kind="ExternalInput")
    rbk_d = nc.dram_tensor("rbk", [128, 4], f32, kind="ExternalInput")
    bv_d = nc.dram_tensor("bv_bc", [128, 512], f32, kind="ExternalInput")
    cos_d = nc.dram_tensor("cosT", [128, T], bf, kind="ExternalInput")
    sin_d = nc.dram_tensor("sinT", [128, T], bf, kind="ExternalInput")
    perm_d = nc.dram_tensor("permT", [128, 128], bf, kind="ExternalInput")
    tri_d = nc.dram_tensor("triadd", [128, 128], f32, kind="ExternalInput")
    yt_d = nc.dram_tensor("yT", [D, T], f32, kind="ExternalOutput")

    x_v = x_d[:, :].rearrange("(ko p) t -> p ko t", p=128)
    wq_v = wq_d[:, :].rearrange("(ko p) m -> p ko m", p=128)
    wk_v = wk_d[:, :].rearrange("(ko p) m -> p ko m", p=128)
    wv_v = wv_d[:, :].rearrange("(ko p) m -> p ko m", p=128)
    wo_v = wo_d[:, :].rearrange("(ko p) n -> p ko n", p=128)

    with tile.TileContext(nc) as tc:
        with (
            tc.tile_pool(name="singles", bufs=1) as singles,
            tc.tile_pool(name="big", bufs=1) as big,
            tc.tile_pool(name="qa_p", bufs=3) as qa_p,
            tc.tile_pool(name="t1_p", bufs=3) as t1_p,
            tc.tile_pool(name="t2_p", bufs=3) as t2_p,
            tc.tile_pool(name="at_p", bufs=7) as at_p,
            tc.tile_pool(name="sp_p", bufs=4) as sp_p,
            tc.tile_pool(name="spb_p", bufs=4) as spb_p,
            tc.tile_pool(name="rbf_p", bufs=4) as rbf_p,
            tc.tile_pool(name="osb_p", bufs=4) as osb_p,
            tc.tile_pool(name="ytmp_p", bufs=2) as ytmp_p,
            tc.tile_pool(name="orow_p", bufs=4) as orow_p,
            tc.tile_pool(name="sps", bufs=2, space="PSUM") as sps,
            tc.tile_pool(name="ops", bufs=2, space="PSUM") as ops,
            tc.tile_pool(name="trans", bufs=2, space="PSUM") as trans,
        ):
            ones1 = singles.tile([1, 64], bf)
            nc.vector.memset(ones1, 1.0)
            bq_sb = singles.tile([128, 4], f32, tag="bq")
            nc.sync.dma_start(bq_sb, bq_d[:, :])
            bk_sb = singles.tile([128, 4], f32, tag="bk")
            nc.sync.dma_start(bk_sb, bk_d[:, :])
            rbq_sb = singles.tile([128, 4], f32, tag="rbq")
            nc.sync.dma_start(rbq_sb, rbq_d[:, :])
            rbk_sb = singles.tile([128, 4], f32, tag="rbk")
            nc.sync.dma_start(rbk_sb, rbk_d[:, :])
            perm_sb = singles.tile([128, 128], bf, tag="perm")
            nc.sync.dma_start(perm_sb, perm_d[:, :])
            tri_sb = singles.tile([128, 128], f32, tag="tri")
            nc.sync.dma_start(tri_sb, tri_d[:, :])

            # persistent activations
            xt = big.tile([128, 8, T], bf, tag="xt")
            qt = big.tile([128, 4, T], bf, tag="qt")
            kt = big.tile([128, 4, T], bf, tag="kt")
            v_sb = big.tile([128, 16, HPC, 65], bf, tag="v")
            nc.gpsimd.memset(v_sb[:, :, :, 64:65], 1.0)
            yt = big.tile([128, 4, T], bf, tag="yt")

            # weights fully resident (chunk-major projections need all of
            # wq/wk at once); wv first so V of strip 0 starts earliest
            wv_sb = big.tile([128, 8, 512], bf, tag="wv")
            nc.scalar.dma_start(wv_sb, wv_v)
            bv_sb = big.tile([128, 512], f32, tag="bv")
            nc.scalar.dma_start(bv_sb, bv_d[:, :])
            wq_sb = big.tile([128, 8, 512], bf, tag="wq")
            nc.scalar.dma_start(wq_sb, wq_v)
            cos_sb = big.tile([128, T], bf, tag="cos")
            nc.scalar.dma_start(cos_sb, cos_d[:, :])
            sin_sb = big.tile([128, T], bf, tag="sin")
            nc.scalar.dma_start(sin_sb, sin_d[:, :])
            wk_sb = big.tile([128, 8, 512], bf, tag="wk")
            nc.scalar.dma_start(wk_sb, wk_v)
            wo_sb = big.tile([128, 4, D], bf, tag="wo")
            nc.scalar.dma_start(wo_sb, wo_v)
            for cc in range(4):
                nc.sync.dma_start(
                    xt[:, :, cc * 512:(cc + 1) * 512],
                    x_v[:, :, cc * 512:(cc + 1) * 512])

            # ---------------- job definitions --------------------------
            proj_tails = []

            def flush_proj_tails():
                for fn in proj_tails:
                    fn()
                proj_tails.clear()

            def proj_v(tt):
                flush_proj_tails()
                psV = trans.tile([128, 512], f32, tag="trans",
                                 name=f"psV_{tt}")
                for ko in range(8):
                    nc.tensor.matmul(
                        psV, lhsT=xt[:, ko, tt * 128:(tt + 1) * 128],
                        rhs=wv_sb[:, ko, :],
                        start=(ko == 0), stop=(ko == 7))

                def tail(psV=psV, tt=tt):
                    nc.vector.tensor_tensor(
                        v_sb[:, tt, :, 0:64],
                        psV.rearrange("p (h d) -> p h d", h=HPC),
                        bv_sb.rearrange("p (h d) -> p h d", h=HPC),
                        Alu.add)

                proj_tails.append(tail)

            def proj_qk(di, qo, cc):
                flush_proj_tails()
                w_sb = wq_sb if di == 0 else wk_sb
                bcol = bq_sb if di == 0 else bk_sb
                rbcol = rbq_sb if di == 0 else rbk_sb
                dest = qt if di == 0 else kt
                c0 = cc * 512
                psA = trans.tile([128, 512], f32, tag="trans",
                                 name=f"psA_{di}_{qo}_{cc}")
                for ko in range(8):
                    nc.tensor.matmul(
                        psA, lhsT=w_sb[:, ko, qo * 128:(qo + 1) * 128],
                        rhs=xt[:, ko, c0:c0 + 512],
                        start=(ko == 0), stop=(ko == 7))
                qa0 = qa_p.tile([128, 512], bf, tag="qa")
                nc.scalar.copy(qa0, psA)

                def tail(psA=psA, qa0=qa0, bcol=bcol, rbcol=rbcol,
                         dest=dest, qo=qo, c0=c0):
                    # tmp1 reads psA before the rot matmul can recycle its
                    # transient-pool slot
                    tmp1 = t1_p.tile([128, 512], bf, tag="tmp1")
                    nc.vector.scalar_tensor_tensor(
                        tmp1, psA, bcol[:, qo:qo + 1],
                        cos_sb[:, c0:c0 + 512], Alu.add, Alu.mult)
                    rps = trans.tile([128, 512], f32, tag="trans",
                                     name=f"rot_{c0}_{qo}")
                    nc.tensor.matmul(
                        rps, lhsT=perm_sb, rhs=qa0, start=True, stop=True)
                    tmp2 = t2_p.tile([128, 512], bf, tag="tmp2")
                    nc.vector.scalar_tensor_tensor(
                        tmp2, rps, rbcol[:, qo:qo + 1],
                        sin_sb[:, c0:c0 + 512], Alu.add, Alu.mult)
                    nc.gpsimd.tensor_tensor(
                        dest[:, qo, c0:c0 + 512], tmp1, tmp2, Alu.add)

                proj_tails.append(tail)

            def outproj_piece(cj, do):
                flush_proj_tails()
                p0 = cj * 512
                ps2 = sps.tile([128, 1024], f32, tag="sps",
                               name=f"op_{cj}_{do}")
                ps = ps2[:, 0:512]
                for ko in range(4):
                    nc.tensor.matmul(
                        ps, lhsT=wo_sb[:, ko, do * 128:(do + 1) * 128],
                        rhs=yt[:, ko, p0:p0 + 512],
                        start=(ko == 0), stop=(ko == 3))
                orow = orow_p.tile([128, 512], f32, tag="orow")
                nc.vector.tensor_copy(orow, ps)
                nc.sync.dma_start(
                    yt_d[do * 128:(do + 1) * 128, p0:p0 + 512], orow)

            def proj_jobs(cc):
                jobs = []
                for j in range(4):
                    jobs.append(lambda tt=4 * cc + j: proj_v(tt))
                for di in range(2):
                    for qo in range(4):
                        jobs.append(
                            lambda di=di, qo=qo, cc=cc: proj_qk(di, qo, cc))
                return jobs

            inject_q = []

            def inject_one():
                if inject_q:
                    inject_q.pop(0)()

            # ---------------- attention machinery ----------------------
            pending = []
            pending_avs = []
            pending_recips = []

            def flush_avs():
                for fn in pending_avs:
                    fn()
                pending_avs.clear()
                for fn in pending_recips:
                    fn()
                pending_recips.clear()

            def flush_tails():
                flush_avs()
                for fn in pending:
                    fn()
                pending.clear()

            def attn_block(ci, ho, stride):
                q0 = ci * 512
                nkt = 4 * ci + 4
                o_pair = [
                    ops.tile([65, 512], f32, tag="ops",
                             name=f"ops_{ci}_{ho}_{g_}")
                    for g_ in range(2)]
                at2s = {}

                def do_av(k_i, o_pair=o_pair, at2s=at2s, nkt=nkt,
                          ho=ho, ci=ci):
                    at2 = at2s.pop(k_i)
                    sdx = k_i - 4 * ci
                    f0 = max(0, sdx) * 128
                    for g in range(2):
                        nc.tensor.matmul(
                            o_pair[g][:, f0:512],
                            lhsT=v_sb[:, k_i, 2 * ho + g, :],
                            rhs=at2[:, g * 512 + f0:(g + 1) * 512],
                            start=(k_i == 0), stop=(k_i == nkt - 1))

                for kt_i in range(nkt):
                    sdx = kt_i - 4 * ci
                    f0 = max(0, sdx) * 128
                    s_ps2 = sps.tile([128, 1024], f32, tag="sps")
                    for g in range(2):
                        hp = g * 64
                        nc.tensor.matmul(
                            s_ps2[:, g * 512 + f0:(g + 1) * 512],
                            lhsT=kt[hp:hp + 64, ho,
                                    kt_i * 128:(kt_i + 1) * 128],
                            rhs=qt[hp:hp + 64, ho, q0 + f0:q0 + 512],
                            start=True, stop=True)
                    at2 = at_p.tile([128, 1024], bf, tag="at")
                    av = at2.rearrange("p (g q) -> p g q", g=2)
                    sv = s_ps2.rearrange("p (g q) -> p g q", g=2)
                    if sdx >= 0:
                        nc.vector.tensor_tensor(
                            sv[:, :, f0:f0 + 128],
                            sv[:, :, f0:f0 + 128],
                            tri_sb[:, None, :].to_broadcast((128, 2, 128)),
                            Alu.add)
                    nc.scalar.activation(
                        av[:, :, f0:512], sv[:, :, f0:512],
                        Act.Exp, scale=0.125)
                    at2s[kt_i] = at2
                    if kt_i == 0:
                        flush_avs()
                    elif kt_i == 1:
                        flush_tails()
                    if kt_i >= LAG:
                        do_av(kt_i - LAG)
                    if stride and kt_i % stride == stride - 1:
                        inject_one()
                for k_i in range(max(0, nkt - LAG), nkt):
                    pending_avs.append(
                        lambda k_i=k_i, do_av=do_av: do_av(k_i))

                for g in range(2):
                    o_ps = o_pair[g]
                    sp = sp_p.tile([16, 32], bf, tag="sp",
                                   name=f"sp_{ci}_{ho}_{g}")
                    o_sb = osb_p.tile([65, 512], bf, tag="osb",
                                      name=f"osb_{ci}_{ho}_{g}")

                    def recip(o_ps=o_ps, sp=sp, o_sb=o_sb):
                        # numerator+denominator leave PSUM in one copy (bank
                        # frees); denominator row spread over 16 partitions
                        nc.vector.tensor_copy(o_sb, o_ps)
                        nc.sync.dma_start(sp, o_sb[64:65, :])

                    pending_recips.append(recip)

                    def rest(g=g, sp=sp, o_sb=o_sb, ho=ho, q0=q0, ci=ci):
                        rsp = sp_p.tile([16, 32], f32, tag="rsp")
                        nc.vector.reciprocal(rsp, sp)
                        rsb = spb_p.tile([16, 32], bf, tag="rsb")
                        nc.vector.tensor_copy(rsb, rsp)
                        rbf = rbf_p.tile([1, 512], bf, tag="rbf")
                        nc.sync.dma_start(rbf, rsb)
                        b_ps = trans.tile([64, 512], f32, tag="trans",
                                          name=f"bps_{ci}_{ho}_{g}")
                        nc.tensor.matmul(
                            b_ps, lhsT=ones1, rhs=rbf,
                            start=True, stop=True)
                        if g == 0:
                            nc.vector.tensor_tensor(
                                yt[0:64, ho, q0:q0 + 512],
                                o_sb[0:64, :], b_ps, Alu.mult)
                        else:
                            ytmp = ytmp_p.tile([64, 512], bf, tag="ytmp")
                            nc.vector.tensor_tensor(
                                ytmp, o_sb[0:64, :], b_ps, Alu.mult)
                            nc.sync.dma_start(
                                yt[64:128, ho, q0:q0 + 512], ytmp)

                    pending.append(rest)

            # ---------------- emission ---------------------------------
            for job in proj_jobs(0):
                job()

            for cc in range(4):
                inject_q.extend(proj_jobs(cc + 1) if cc < 3 else [])
                for ho in range(4):
                    if ho == 1 and cc > 0:
                        # yt(cc-1) final tails flushed during (cc, ho=0)
                        inject_q.extend(
                            [lambda cj=cc - 1, do=do_:
                             outproj_piece(cj, do)
                             for do_ in range(8)])
                    njobs = len(inject_q) + (8 if (ho == 0 and cc > 0)
                                             else 0)
                    tiles_left = (4 - ho) * (4 * cc + 4)
                    stride = max(1, tiles_left // max(1, njobs)) \
                        if njobs else 0
                    attn_block(cc, ho, stride)
                while inject_q:
                    inject_one()

            flush_tails()
            flush_proj_tails()
            for do in range(8):
                outproj_piece(3, do)

    nc.finalize()
    return nc


def _prep_shards(x, Wq, bq, Wk, bk, Wv, bv, Wo, bo):
    import ml_dtypes

    f = np.float32
    bft = ml_dtypes.bfloat16
    theta = 1.0 / (ROPE_BASE ** (np.arange(0, HD, 2, dtype=f) / HD))  # [32]
    pos = np.arange(1, T + 1, dtype=f)
    ang = pos[:, None] * theta[None, :]  # [T, 32]
    j = (np.arange(128) % HD) % 32
    cosT = np.ascontiguousarray(np.cos(ang).T[j, :].astype(bft))  # [128, T]
    sinT = np.ascontiguousarray(np.sin(ang).T[j, :].astype(bft))
    # rotate-half permutation (with sign): rot[p] = sgn(p) * q[p ^ 32]
    prm = np.zeros((128, 128), dtype=f)
    pp = np.arange(128)
    prm[pp, pp ^ 32] = np.where((pp % HD) < 32, -1.0, 1.0)
    permT = np.ascontiguousarray(prm.T.astype(bft))

    # additive causal mask for the diagonal 128-block: keep c >= p
    cc = np.arange(128)[None, :]
    triadd = np.where(cc >= pp[:, None], 0.0, -1e30).astype(f)
    triadd = np.ascontiguousarray(triadd)

    def col128(b_):  # [512] -> [128, 4] (partition-major per 128-tile)
        return np.ascontiguousarray(np.asarray(b_, dtype=f).reshape(4, 128).T)

    in_maps = []
    for c in range(N_CORES):
        b, hg = c // 2, c % 2
        sl = slice(hg * 512, hg * 512 + 512)
        bqc, bkc = col128(bq[sl]), col128(bk[sl])
        in_maps.append({
            "xT": np.ascontiguousarray(np.asarray(x[b], dtype=f).T
                                       .astype(bft)),
            "wq": np.ascontiguousarray(np.asarray(Wq[sl, :], dtype=f).T
                                       .astype(bft)),
            "wk": np.ascontiguousarray(np.asarray(Wk[sl, :], dtype=f).T
                                       .astype(bft)),
            "wv": np.ascontiguousarray(np.asarray(Wv[sl, :], dtype=f).T
                                       .astype(bft)),
            "wo": np.ascontiguousarray(np.asarray(Wo[:, sl], dtype=f).T
                                       .astype(bft)),
            "bq": bqc, "bk": bkc,
            "rbq": np.ascontiguousarray(prm @ bqc),
            "rbk": np.ascontiguousarray(prm @ bkc),
            "bv_bc": np.ascontiguousarray(
                np.tile(np.asarray(bv[sl], dtype=f)[None, :], (128, 1))),
            "cosT": cosT, "sinT": sinT,
            "permT": permT, "triadd": triadd,
        })
    return in_maps


def _run(inputs, trace=False):
    from concourse import bass_utils

    if "nc" not in _cache:
        _cache["nc"] = _build_bass()
    nc = _cache["nc"]
    in_maps = _prep_shards(**inputs)
    # The remote device occasionally reports a transient unrecoverable
    # state right after loading a fresh NEFF; a retry reliably clears it.
    last_exc = None
    for _ in range(3):
        try:
            res = bass_utils.run_bass_kernel_spmd(
                nc, in_maps, core_ids=list(range(N_CORES)), trace=trace)
            break
        except Exception as e:  # noqa: BLE001
            last_exc = e
            import time
            time.sleep(2.0)
    else:
        raise last_exc

    bo = np.asarray(inputs["bo"], dtype=np.float32)
    out = np.empty((B, T, D), dtype=np.float32)
    for b in range(B):
        out[b] = (res.results[2 * b]["yT"].T
                  + res.results[2 * b + 1]["yT"].T + bo)
    return out, res


def kernel(**inputs):
    out, _ = _run(inputs, trace=False)
    return out
